# revision 39
# baseline (speedup 1.0000x reference)
"""Trainium2 Bass kernel for the masked MQA attention block (nn_Attention_4252017623134).

Sharding: pure data-parallel over batch. b=8 batch elements, 8 NeuronCores,
one batch element per core, weights replicated. No collectives.

Per-core math (n=1024, d=1024, h=16, dh=64, inner=1024):
  context = x                      (pre-norm residual branch feeds K/V)
  xn  = layernorm(x) * g_in
  q   = xn @ Wq.T   (per head, scaled by 1/8 = dh^-0.5, folded into exp scale)
  k,v = context @ Wkv.T (single shared KV head) + null_kv token
  att = softmax(mask(q k^T / 8))   (padding + causal(key j visible iff j <= i))
  out = layernorm(att @ v @ Wo.T) * g_out

Key design decisions:
  * All matmuls in bf16 (1 PE row/cycle at ANY width, vs f32r needing >=256).
  * LN1 folded into the q-projection: q_i = r_i * (Wq'' @ x_i) with
    Wq'' = Wq*diag(g) - outer(Wq@g, 1)/D precomputed on HOST (mean removal)
    and r_i = rsqrt(var_i+eps) applied as a per-column scale at PSUM evac.
    Removes all xn transposes and the LN->transpose->matmul serialization.
  * Null-token scores are 16 extra projection channels (wn = nk . Wq''_h,
    host-computed) -> no separate null-score machinery.
  * Padding mask applied by ZEROING masked k/v columns instead of an exp
    bias: masked j gives u=exp(0)=1 but contributes v_j=0 to the numerator
    and is excluded from the denominator via a mask column appended to V.
    Exp needs NO per-partition bias, so one activation call spans the score
    tiles of a whole multi-j-tile group.
  * Scores computed transposed (simT[j,i], exact visible windows); P@V runs
    NATURAL: lhsT = u[j, i-chunk], rhs = v_nat [j, 64ch + mask col]. Each
    accumulation step costs 65 cycles instead of an i-window: PV ~19us
    instead of ~37us. Softmax denominator lands in PSUM col 64; division is
    fused into the PV evac as a per-partition reciprocal multiply.
  * PV output [i, c] returns to [c, i] via XBAR DMA transposes (no PE/DVE).
  * Engine choreography: weight DMAs split across the SP and Activation
    HWDGE queues; per-pair emission interleaves q-proj -> scores -> PV so
    the Act engine (exp, the phase-B bottleneck) starts at ~14us and never
    starves; kT half-duplication via SBUF->SBUF DMA keeps Copy off the Act
    activation tables.
"""

import contextlib

import numpy as np
import ml_dtypes

import concourse.bass as bass
import concourse.bacc as bacc
import concourse.tile as tile
import concourse.mybir as mybir
from concourse.bass_utils import run_bass_kernel_spmd
from concourse.masks import make_identity

N = 1024          # sequence length per core
D = 1024          # model dim
H = 16            # query heads
DH = 64           # head dim
INNER = H * DH    # 1024
NT = N // 128     # 8 i-tiles / j-tiles / d-tiles
EPS = 1e-5

F32 = mybir.dt.float32
BF16 = mybir.dt.bfloat16
U8 = mybir.dt.uint8
AF = mybir.ActivationFunctionType
ALU = mybir.AluOpType

# exp groups per head: j-tiles packed into [128, 1024] fp32 (2-bank) PSUM
# tiles; every matmul segment stays inside a 512-col bank.
EXP_GROUPS = [(0,), (1, 7), (2, 6), (3, 5), (4,)]
U_OFF = {}
_off = 0
for _g in EXP_GROUPS:
    for _t in _g:
        U_OFF[_t] = _off
        _off += N - 128 * _t
U_COLS = _off  # 4608


def _bank_chunks(c0, c1):
    """Split [c0, c1) at 512-column (2KB fp32 PSUM bank) boundaries."""
    out = []
    while c0 < c1:
        nxt = min(c1, (c0 // 512 + 1) * 512)
        out.append((c0, nxt))
        c0 = nxt
    return out


def _emit(nc):
    # ---------------- DRAM I/O ----------------
    xT_d = nc.dram_tensor("xT", [D, N], BF16, kind="ExternalInput")
    wqT_d = nc.dram_tensor("wqT", [D, INNER], BF16, kind="ExternalInput")
    wnT_d = nc.dram_tensor("wnT", [D, H], BF16, kind="ExternalInput")
    wkvT_d = nc.dram_tensor("wkvT", [D, 2 * DH], BF16, kind="ExternalInput")
    woT_d = nc.dram_tensor("woT", [INNER, D], BF16, kind="ExternalInput")
    vnull_d = nc.dram_tensor("vnull", [DH], F32, kind="ExternalInput")
    mask_d = nc.dram_tensor("mask", [N], U8, kind="ExternalInput")
    gout_d = nc.dram_tensor("gout", [D], F32, kind="ExternalInput")
    out_d = nc.dram_tensor("out", [N, D], F32, kind="ExternalOutput")

    d_ = dict(xT_d=xT_d, wqT_d=wqT_d, wnT_d=wnT_d, wkvT_d=wkvT_d,
              woT_d=woT_d, vnull_d=vnull_d, mask_d=mask_d, gout_d=gout_d,
              out_d=out_d)
    with tile.TileContext(nc) as tc:
        _emit_tile(nc, tc, d_)
    return nc


def _emit_tile(nc, tc, d_):
    xT_d = d_["xT_d"]
    wqT_d, wnT_d, wkvT_d, woT_d = d_["wqT_d"], d_["wnT_d"], d_["wkvT_d"], d_["woT_d"]
    vnull_d, mask_d = d_["vnull_d"], d_["mask_d"]
    gout_d, out_d = d_["gout_d"], d_["out_d"]

    ctx = contextlib.ExitStack()
    with ctx:
        consts = ctx.enter_context(tc.tile_pool(name="consts", bufs=1))
        persist = ctx.enter_context(tc.tile_pool(name="persist", bufs=1))
        stage = ctx.enter_context(tc.tile_pool(name="stage", bufs=4))

        # ------------- persistent tiles -------------
        kT2 = persist.tile([128, N], BF16, tag="kT2")       # k^T in both halves
        v_nat = persist.tile([128, NT, DH + 1], BF16, tag="v_nat")  # col 64 = mask
        vnull16 = persist.tile([1, DH + 1], BF16, tag="vnull16")    # [v_null | 1]
        unull = persist.tile([H, N], BF16, tag="unull")     # null-token exp rows
        unull_r = persist.tile([1, H * N], BF16, tag="unull_r")  # partition-0 form
        outT = persist.tile([128, NT, NT, 128], BF16, tag="outT")  # [c-pair, it, i]
        rbroad = persist.tile([128, N], F32, tag="rbroad")  # rstd row broadcast
        qT = persist.tile([128, NT, N], BF16, tag="qT")     # q^T (pair slabs)
        xTs = persist.tile([128, NT, N], BF16, tag="xTs")   # x^T resident
        wqs = persist.tile([128, NT, INNER], BF16, tag="wqs")  # reused for Wo in C
        wkvs = persist.tile([128, NT, 2 * DH], BF16, tag="wkvs")
        wns = persist.tile([128, NT, H], BF16, tag="wns")
        vts = persist.tile([128, N], BF16, tag="vts")   # rows 64:128 = masked v^T
        gout_b = persist.tile([128, D], F32, tag="gout_b")

        # ---- DMA issue: Act HWDGE queue (xT t0-3, wn, wq t4-7) ----
        for t in range(4):
            nc.scalar.dma_start(out=xTs[:, t, :], in_=xT_d[t * 128:(t + 1) * 128, :])
        nc.scalar.dma_start(out=wns[:],
                            in_=wnT_d.ap().rearrange("(t p) c -> p t c", p=128))
        for t in range(4, NT):
            nc.scalar.dma_start(out=wqs[:, t, :],
                                in_=wqT_d[t * 128:(t + 1) * 128, :])

        # ---- DMA issue: SP HWDGE queue (wkv, xT t4-7, masks, wq t0-3) ----
        nc.sync.dma_start(out=wkvs[:],
                          in_=wkvT_d.ap().rearrange("(t p) c -> p t c", p=128))
        for t in range(4, NT):
            nc.sync.dma_start(out=xTs[:, t, :], in_=xT_d[t * 128:(t + 1) * 128, :])
        vn_s = stage.tile([1, DH], F32, tag="vn")
        nc.sync.dma_start(out=vn_s[:],
                          in_=bass.AP(tensor=vnull_d, offset=0,
                                      ap=[[0, 1], [1, DH]]))
        maskb_u8 = consts.tile([128, N], U8)
        nc.sync.dma_start(out=maskb_u8[:],
                          in_=bass.AP(tensor=mask_d, offset=0,
                                      ap=[[0, 128], [1, N]]))
        maskc_u8 = consts.tile([128, NT], U8)
        nc.sync.dma_start(out=maskc_u8[:],
                          in_=bass.AP(tensor=mask_d, offset=0,
                                      ap=[[1, 128], [128, NT]]))
        for t in range(4):
            nc.sync.dma_start(out=wqs[:, t, :],
                              in_=wqT_d[t * 128:(t + 1) * 128, :])
        nc.sync.dma_start(out=gout_b[:],
                          in_=bass.AP(tensor=gout_d, offset=0,
                                      ap=[[0, 128], [1, D]]))

        # ---------------- constants ----------------
        ident = consts.tile([128, 128], BF16)
        make_identity(nc, ident[:])
        # causal 0/1 band mask: keep u[j_rel, i_rel] iff i_rel >= j_rel
        mtri = consts.tile([128, 128], BF16)
        nc.gpsimd.memset(mtri[:], 1.0)
        nc.gpsimd.affine_select(out=mtri[:], in_=mtri[:], compare_op=ALU.is_ge,
                                fill=0.0, base=0, pattern=[[1, 128]],
                                channel_multiplier=-1)
        # mask converts + v_nat plumbing live on gpsimd (DVE stays on the
        # squares -> var critical chain)
        maskb = consts.tile([128, N], BF16)
        nc.gpsimd.tensor_copy(maskb[:], maskb_u8[:])
        maskc = consts.tile([128, NT], BF16)
        nc.gpsimd.tensor_copy(maskc[:], maskc_u8[:])
        eps_t = consts.tile([128, 1], F32)
        nc.vector.memset(eps_t[:], EPS)
        ones_t = consts.tile([128, 2], BF16)
        nc.vector.memset(ones_t[:], 1.0)
        # warm the ACT tables (Sqrt/Exp) outside any dependency chain
        warm = consts.tile([128, 2], F32)
        nc.scalar.activation(out=warm[:, 0:1], in_=eps_t[:], func=AF.Sqrt)
        nc.scalar.activation(out=warm[:, 1:2], in_=eps_t[:], func=AF.Exp)

        # ============ Phase A: projections + matmul-based LN1 stats ============
        # One 8-bank PSUM scope: pkv(2) sx(2) sx2(2) pnull(2). Stats matmuls
        # lead the PE stream (the rstd chain is the long serial pole), then
        # kv, then null scores. V-transposes go through the XBAR DMA engine.
        with tc.tile_pool(name="psA", bufs=1, space="PSUM") as psA:
            pkv = psA.tile([128, N], F32, tag="pkv")
            sx = psA.tile([1, N], F32, tag="sx")
            sx2 = psA.tile([1, N], F32, tag="sx2")
            pnull = psA.tile([H, N], F32, tag="pnull")
            for t in range(NT):
                x2 = stage.tile([128, N], BF16, tag="x2", name="x2", bufs=4)
                nc.vector.tensor_mul(x2[:], xTs[:, t, :], xTs[:, t, :])
                for ch in range(2):
                    cs = slice(ch * 512, (ch + 1) * 512)
                    nc.tensor.matmul(sx[:, cs], ones_t[:, 0:1], xTs[:, t, cs],
                                     start=(t == 0), stop=(t == NT - 1))
                    nc.tensor.matmul(sx2[:, cs], ones_t[:, 0:1], x2[:, cs],
                                     start=(t == 0), stop=(t == NT - 1))
            for t in range(NT):
                for ch in range(2):
                    cs = slice(ch * 512, (ch + 1) * 512)
                    nc.tensor.matmul(pkv[:, cs], wkvs[:, t, :], xTs[:, t, cs],
                                     start=(t == 0), stop=(t == NT - 1))
            for t in range(NT):
                for ch in range(2):
                    cs = slice(ch * 512, (ch + 1) * 512)
                    nc.tensor.matmul(pnull[:, cs], wns[:, t, :], xTs[:, t, cs],
                                     start=(t == 0), stop=(t == NT - 1))

            # var*D = sx2 - sx^2/D ; rstd = rsqrt(var + eps) broadcast to rows
            t1 = stage.tile([1, N], F32, tag="t1", bufs=1)
            sxs = stage.tile([1, N], F32, tag="sxs", bufs=1)
            nc.vector.tensor_copy(sxs[:], sx[:])
            nc.vector.scalar_tensor_tensor(out=t1[:], in0=sxs[:], scalar=1.0 / D,
                                           in1=sxs[:], op0=ALU.mult, op1=ALU.mult)
            nc.vector.tensor_sub(t1[:], sx2[:], t1[:])
            nc.scalar.activation(out=t1[:], in_=t1[:], func=AF.Sqrt,
                                 bias=eps_t[0:1, :], scale=1.0 / D)
            nc.scalar.activation(out=warm[:, 1:2], in_=eps_t[:], func=AF.Exp)

            # evac: masked k^T (low half), masked v^T (rows 64:128)
            nc.vector.scalar_tensor_tensor(out=kT2[0:64, :], in0=pkv[0:64, :],
                                           scalar=1.0, in1=maskb[0:64, :],
                                           op0=ALU.mult, op1=ALU.mult)
            nc.sync.dma_start(out=kT2[64:128, :], in_=kT2[0:64, :])
            nc.vector.scalar_tensor_tensor(out=vts[64:128, :], in0=pkv[64:128, :],
                                           scalar=1.0, in1=maskb[64:128, :],
                                           op0=ALU.mult, op1=ALU.mult)
            r_row = stage.tile([1, N], F32, tag="rrow", bufs=1)
            nc.vector.reciprocal(out=r_row[:], in_=t1[:])
            nc.gpsimd.partition_broadcast(rbroad[:], r_row[:])

            # (v transposes happen on the PE inside the B loop at m==0, when
            #  the transpose-staging PSUM pool exists; XBAR DMA transpose is
            #  broken for nonzero AP offsets)
            for t in range(NT):
                nc.gpsimd.tensor_copy(v_nat[:, t, DH:DH + 1], maskc[:, t:t + 1])
            nc.gpsimd.tensor_copy(vnull16[:, 0:DH], vn_s[:])
            nc.gpsimd.tensor_copy(vnull16[:, DH:DH + 1], ones_t[0:1, 0:1])

            # --- null exp: unull[h,i] = exp(0.125 * r_i * pnull) ---
            nl_s = stage.tile([H, N], F32, tag="nls", bufs=1)
            nc.vector.scalar_tensor_tensor(out=nl_s[:], in0=pnull[:],
                                           scalar=1.0, in1=rbroad[0:H, :],
                                           op0=ALU.mult, op1=ALU.mult)
            nc.scalar.activation(out=unull[:], in_=nl_s[:], func=AF.Exp,
                                 scale=0.125)
            # reshape to partition 0 (matmul lhsT base must be 0/32/64/96)
            nc.sync.dma_start(out=unull_r[:], in_=unull[:])

        if True:
            # ============ Phase B: q-proj + attention, per head pair ============
            with tc.tile_pool(name="psS", bufs=2, space="PSUM") as psS, \
                 tc.tile_pool(name="psPV", bufs=2, space="PSUM") as psPV, \
                 tc.tile_pool(name="psQ", bufs=1, space="PSUM") as psQ, \
                 tc.tile_pool(name="psTR", bufs=1, space="PSUM") as psTR, \
                 tc.tile_pool(name="upool", bufs=4) as upool, \
                 tc.tile_pool(name="opool", bufs=2) as opool, \
                 tc.tile_pool(name="rcpool", bufs=4) as rcpool:

                def pv_block(m, us):
                    o_nat = opool.tile([128, NT, 128], BF16, tag="onat",
                                       name="o_nat")
                    ptr = psTR.tile([128, NT, 128], BF16, tag="ptr", name="ptr")
                    for half in range(2):
                        for ph in range(2):
                            h = 2 * m + ph
                            base = 64 * ph
                            u = us[ph]
                            pv = psPV.tile([128, 4, DH + 1], F32, tag="pv",
                                           name="pv")
                            for q_ in range(4):
                                it = 4 * half + q_
                                for t in range(it + 1):
                                    uo = U_OFF[t] + 128 * (it - t)
                                    nc.tensor.matmul(pv[:, q_, :],
                                                     u[:, uo:uo + 128],
                                                     v_nat[:, t, :],
                                                     start=(t == 0), stop=False)
                                nc.tensor.matmul(
                                    pv[:, q_, :],
                                    unull_r[0:1,
                                            h * N + it * 128:h * N + it * 128 + 128],
                                    vnull16[0:1, :],
                                    start=False, stop=True)
                            rc = rcpool.tile([128, 4, 1], F32, tag="rc",
                                             name="rc")
                            nc.vector.reciprocal(out=rc[:], in_=pv[:, :, DH:DH + 1])
                            nc.vector.scalar_tensor_tensor(
                                out=o_nat[:, 4 * half:4 * half + 4,
                                          base:base + 64],
                                in0=pv[:, :, 0:DH], scalar=1.0,
                                in1=rc[:].broadcast_to([128, 4, DH]),
                                op0=ALU.mult, op1=ALU.mult)
                        for q_ in range(4):
                            it = 4 * half + q_
                            nc.tensor.transpose(ptr[:, it, :], o_nat[:, it, :],
                                                ident[:])
                    nc.vector.tensor_copy(outT[:, m, :, :], ptr[:])

                prev = None
                for m in range(NT):              # head pairs
                    # --- q-projection for this pair ---
                    ms = slice(m * 128, (m + 1) * 128)
                    for ch in range(2):
                        pq = psQ.tile([128, 512], F32, tag="pq", name="pq")
                        for t in range(NT):
                            nc.tensor.matmul(pq[:], wqs[:, t, ms],
                                             xTs[:, t, ch * 512:(ch + 1) * 512],
                                             start=(t == 0), stop=(t == NT - 1))
                        nc.vector.scalar_tensor_tensor(
                            out=qT[:, m, ch * 512:(ch + 1) * 512], in0=pq[:],
                            scalar=1.0, in1=rbroad[:, ch * 512:(ch + 1) * 512],
                            op0=ALU.mult, op1=ALU.mult)
                    if m == 0:
                        # v -> natural [j, c] tiles (PE transposes, staged in
                        # a psTR-shaped tile, one strided evac)
                        pvt = psTR.tile([128, NT, 128], BF16, tag="ptr",
                                        name="pvt")
                        for t in range(NT):
                            nc.tensor.transpose(pvt[:, t, 0:DH],
                                                vts[64:128, t * 128:(t + 1) * 128],
                                                ident[64:128, 64:128])
                        nc.vector.tensor_copy(v_nat[:, :, 0:DH],
                                              pvt[:, :, 0:DH])
                    if m == NT - 1:
                        # wqs is dead after this pair's q-proj: refill with Wo
                        for t in range(NT):
                            nc.sync.dma_start(out=wqs[:, t, :],
                                              in_=woT_d[t * 128:(t + 1) * 128, :])
                    # --- scores + exp (both parities: keeps Act fed) ---
                    us = []
                    for ph in range(2):
                        base = 64 * ph
                        u = upool.tile([128, U_COLS], BF16, tag="u", name="u")
                        us.append(u)
                        for grp in EXP_GROUPS:
                            ps = psS.tile([128, N], F32, tag="scores",
                                          name="ps")
                            goff = U_OFF[grp[0]]
                            for t in grp:
                                lo = 128 * t
                                co = U_OFF[t] - goff
                                for c0, c1 in _bank_chunks(co, co + N - lo):
                                    nc.tensor.matmul(
                                        ps[:, c0:c1],
                                        kT2[base:base + 64, lo:lo + 128],
                                        qT[base:base + 64, m,
                                           lo + (c0 - co):lo + (c1 - co)],
                                        start=True, stop=True)
                            gw = sum(N - 128 * t for t in grp)
                            nc.scalar.activation(out=u[:, goff:goff + gw],
                                                 in_=ps[:, 0:gw], func=AF.Exp,
                                                 scale=0.125)
                            for t in grp:  # causal band of each tile in group
                                nc.vector.tensor_mul(
                                    u[:, U_OFF[t]:U_OFF[t] + 128],
                                    u[:, U_OFF[t]:U_OFF[t] + 128], mtri[:])
                    pv_block(m, us)

        # ============ Phase C: out-projection + LN2 ============
        with tc.tile_pool(name="psC", bufs=4, space="PSUM") as psC:
            for it in range(NT):
                st = stage.tile([128, 2, 6], F32, tag="bnst")
                pos = []
                for ch in range(2):
                    po = psC.tile([128, 512], F32, tag="po")
                    for ct in range(NT):
                        nc.tensor.matmul(po[:], outT[:, ct, it, :],
                                         wqs[:, ct, ch * 512:(ch + 1) * 512],
                                         start=(ct == 0), stop=(ct == NT - 1))
                    nc.vector.bn_stats(out=st[:, ch, :], in_=po[:])
                    pos.append(po)
                mv = stage.tile([128, 2], F32, tag="bnmv")
                nc.vector.bn_aggr(out=mv[:], in_=st[:])
                rstd = stage.tile([128, 1], F32, tag="rstd")
                nc.scalar.activation(out=rstd[:], in_=mv[:, 1:2], func=AF.Sqrt,
                                     bias=eps_t[:], scale=1.0)
                nc.vector.reciprocal(out=rstd[:], in_=rstd[:])
                o_s = stage.tile([128, D], F32, tag="os", bufs=2)
                for ch in range(2):
                    cs = slice(ch * 512, (ch + 1) * 512)
                    nc.vector.tensor_scalar(out=o_s[:, cs], in0=pos[ch][:],
                                            scalar1=mv[:, 0:1], scalar2=rstd[:],
                                            op0=ALU.subtract, op1=ALU.mult)
                    nc.gpsimd.tensor_mul(o_s[:, cs], o_s[:, cs], gout_b[:, cs])
                    nc.sync.dma_start(out=out_d[it * 128:(it + 1) * 128, cs],
                                      in_=o_s[:, cs])


_CACHED = None


def _get_nc():
    global _CACHED
    if _CACHED is None:
        nc = bacc.Bacc("TRN2", target_bir_lowering=False, debug=False)
        _emit(nc)
        nc.compile()
        _CACHED = nc
    return _CACHED


def make_in_maps(x, mask, g_in, Wq, Wkv, null_kv, Wo, g_out):
    b = x.shape[0]
    BF = ml_dtypes.bfloat16
    g = g_in.astype(np.float64)
    W2 = Wq.astype(np.float64) * g[None, :]                  # [INNER, D]
    Wqq = W2 - W2.sum(axis=1, keepdims=True) / D             # fold mean removal
    wn = np.einsum('k,hkd->hd', null_kv[0].astype(np.float64),
                   Wqq.reshape(H, DH, D))                    # [H, D]
    xT = np.transpose(x, (0, 2, 1))
    mask_u8 = np.ascontiguousarray(mask).view(np.uint8) if mask.dtype == np.bool_ \
        else mask.astype(np.uint8)
    shared = {
        "wqT": np.ascontiguousarray(Wqq.T.astype(BF)),
        "wnT": np.ascontiguousarray(wn.T.astype(BF)),
        "wkvT": np.ascontiguousarray(Wkv.T.astype(BF)),
        "woT": np.ascontiguousarray(Wo.T.astype(BF)),
        "vnull": np.ascontiguousarray(null_kv[1].astype(np.float32)),
        "gout": np.ascontiguousarray(g_out.astype(np.float32)),
    }
    return [
        {"xT": np.ascontiguousarray(xT[c].astype(BF)),
         "mask": mask_u8[c], **shared}
        for c in range(b)
    ]


def kernel(x, mask, g_in, Wq, Wkv, null_kv, Wo, g_out):
    x = np.asarray(x)
    mask = np.asarray(mask)
    g_in, g_out = np.asarray(g_in), np.asarray(g_out)
    Wq, Wkv, Wo = np.asarray(Wq), np.asarray(Wkv), np.asarray(Wo)
    null_kv = np.asarray(null_kv)
    b = x.shape[0]
    assert x.shape == (b, N, D) and b == 8
    in_maps = make_in_maps(x, mask, g_in, Wq, Wkv, null_kv, Wo, g_out)
    nc = _get_nc()
    res = run_bass_kernel_spmd(nc, in_maps, core_ids=list(range(b)))
    return np.stack([res.results[c]["out"] for c in range(b)], axis=0)
